# revision 3
# baseline (speedup 1.0000x reference)
"""Trainium2 kernel for nn_Propagation: 3x3 dilated shifted-patch extraction
(reflect padding) of two [2, 64, 256, 256] f32 tensors -> two
[2, 576, 256, 256] f32 tensors.

Strategy: pure data movement. Shard H across the 8 cores (32 rows each).
The host pre-computes each core's reflect-padded shard (rows + cols padded
by d), so the device kernel is only DMAs: load the padded shard once into
SBUF, then write the 9 shifted window copies per batch straight back to HBM.
Minimal HBM traffic: read 1x input + write 9x input.
"""

import os
import sys

import numpy as np

try:
    import concourse  # noqa: F401
except ImportError:  # make kernel.py self-contained wrt sys.path
    for p in ("/root/.axon_site", "/root/.axon_site/_ro/trn_rl_repo",
              "/root/.axon_site/_ro/pypackages", "/opt/trn_rl_repo"):
        if os.path.isdir(p) and p not in sys.path:
            sys.path.append(p)

import concourse.bass as bass
import concourse.mybir as mybir
from concourse.bass_utils import run_bass_kernel_spmd

N_CORES = 8
B, C, H, W = 2, 64, 256, 256
F = 3  # filter size
ROWS = H // N_CORES  # 32 output rows per core

_cache = {}


def _build_nc(d: int) -> bass.Bass:
    PR = ROWS + 2 * d  # padded rows in a core's shard
    PW = W + 2 * d  # padded width
    f32 = mybir.dt.float32

    nc = bass.Bass("TRN2")
    xs = nc.dram_tensor("xs", [B * C, PR, PW], f32, kind="ExternalInput")
    ys = nc.dram_tensor("ys", [B * C, PR, PW], f32, kind="ExternalInput")
    ox = nc.dram_tensor("ox", [B, F * F * C, ROWS, W], f32, kind="ExternalOutput")
    oy = nc.dram_tensor("oy", [B, F * F * C, ROWS, W], f32, kind="ExternalOutput")

    with (
        nc.sbuf_tensor("tx", [B * C, PR, PW], f32) as tx,
        nc.sbuf_tensor("ty", [B * C, PR, PW], f32) as ty,
        nc.semaphore("xl") as xl_sem,
        nc.semaphore("yl") as yl_sem,
        nc.semaphore("xst") as xs_sem,
        nc.semaphore("yst") as ys_sem,
        nc.Block() as block,
    ):
        # x on the SP HWDGE ring, y on the ACT HWDGE ring — the two rings
        # generate descriptors independently.
        def emit(eng, src, dst, tile, load_sem, store_sem):
            for b in range(B):
                eng.dma_start(
                    out=tile[b * C : (b + 1) * C],
                    in_=src[b * C : (b + 1) * C],
                ).then_inc(load_sem, 16)
            n_store = 0
            for b in range(B):
                eng.wait_ge(load_sem, 16 * (b + 1))
                for i in range(F):
                    for j in range(F):
                        k = i * F + j
                        eng.dma_start(
                            out=dst[b, k * C : (k + 1) * C, :, :],
                            in_=tile[
                                b * C : (b + 1) * C,
                                i * d : i * d + ROWS,
                                j * d : j * d + W,
                            ],
                        ).then_inc(store_sem, 16)
                        n_store += 1
            eng.wait_ge(store_sem, 16 * n_store)

        @block.sync
        def _(sync):
            emit(sync, xs, ox, tx, xl_sem, xs_sem)

        @block.scalar
        def _(scalar):
            emit(scalar, ys, oy, ty, yl_sem, ys_sem)

    return nc


def kernel(inref_x: np.ndarray, inref_y: np.ndarray, dilation) -> tuple:
    d = int(dilation)
    x = np.ascontiguousarray(np.asarray(inref_x, dtype=np.float32))
    y = np.ascontiguousarray(np.asarray(inref_y, dtype=np.float32))

    if d not in _cache:
        _cache[d] = _build_nc(d)
    nc = _cache[d]

    # Host-side halo prep: reflect-pad, then cut per-core row shards.
    px = np.pad(x, ((0, 0), (0, 0), (d, d), (d, d)), mode="reflect")
    py = np.pad(y, ((0, 0), (0, 0), (d, d), (d, d)), mode="reflect")
    PR = ROWS + 2 * d
    PW = W + 2 * d
    in_maps = []
    for m in range(N_CORES):
        r0 = m * ROWS
        in_maps.append(
            {
                "xs": np.ascontiguousarray(
                    px[:, :, r0 : r0 + PR, :].reshape(B * C, PR, PW)
                ),
                "ys": np.ascontiguousarray(
                    py[:, :, r0 : r0 + PR, :].reshape(B * C, PR, PW)
                ),
            }
        )

    res = run_bass_kernel_spmd(nc, in_maps, core_ids=list(range(N_CORES)))

    agg_x = np.concatenate([r["ox"] for r in res.results], axis=2)
    agg_y = np.concatenate([r["oy"] for r in res.results], axis=2)
    return agg_x, agg_y
